# revision 23
# baseline (speedup 1.0000x reference)
"""Trainium2 Bass kernel for a 2-layer GCN + FC head (nn_CNNGNNModel).

Reference computation (PyG GCNConv semantics, symmetric normalization with
self-loops):
    deg[i]  = in-degree(i) + 1 ;  dinv = deg^-0.5
    A_hat   = D^-1/2 (A + I) D^-1/2   (aggregation by destination)
    h1 = relu(A_hat @ (x @ W1) + b1)
    h2 = relu(A_hat @ (h1 @ W2) + b2)
    out = h2 @ Wfc + bfc

The per-edge weight dinv[src]*dinv[dst] is separable: node features are
scaled by dinv on the way out of each matmul (source side) and the
aggregate is scaled by dinv after the segmented sum (dest side), so message
passing is a pure gather + segmented sum.

Distribution (8 NeuronCores, SPMD single program):
  - Nodes are sharded by id range: core c owns dests [c*12500, (c+1)*12500),
    padded to 12544 = 98*128 slots.  All index plumbing is precomputed on
    the host in "position" space pos = owner*12544 + slot.
  - Each layer: local matmul of the core's node block -> dinv-scaled bf16
    features -> AllGather to a full table [100352, 256] bf16 -> dma_gather
    of incoming messages -> DVE segmented sum -> relu -> (fused) transpose
    + next-layer matmul per block.
  - dma_gather indices are int16 (<= 32767); the table is addressed in 4
    residue "chunks" of 25088 rows via the in_ AP base offset.  The HW
    SWDGE limit is 1024 indices per call, so all calls are packed to
    exactly 1024 indices (8 columns x 128 dests): blocks are processed in
    groups of 4, and per (group, chunk) the per-dest message counts are
    padded to a shared even column count l, giving uniform [128, 4, l, 256]
    tiles whose segmented sum is a log2(l) fold-tree of wide strided DVE
    adds.
  - Weights are replicated; output [12544, 1000] bf16 per core is cast to
    f32 and reassembled (inverse permutation) on the host.
"""

import numpy as np
import ml_dtypes

import concourse.bass as bass
import concourse.bacc as bacc
import concourse.mybir as mybir
import concourse.tile as tile
from concourse.bass_utils import run_bass_kernel_spmd
from concourse.masks import make_identity

BF16 = ml_dtypes.bfloat16

N_CORES = 8
N_NODES = 100000
IN_DIM = 512
HID = 256
NCLS = 1000
NLOC = 12500          # real dests per core
SLOTS = 12544         # padded dests per core (98 blocks of 128)
BLOCKS = SLOTS // 128  # 98
NCHUNK = 4
CH_ROWS = 2 * SLOTS   # 25088 rows per chunk (= 2 cores)
ZERO_LOCAL = 12543    # chunk-local row guaranteed to be a zero pad row
P = 128
GMAX = 4              # blocks per group
LMAX = 12             # max fold columns per sub-piece (even)
CALL_COLS = 8         # 8 cols * 128 = 1024 idx per dma_gather (HW limit)
# tile-pool depths (pipelining knobs)
XT_BUFS = 2
HST_BUFS = 2
IDX_BUFS = 3
PC_BUFS = 5
ACC_BUFS = 3
HFIN_BUFS = 2
HT_BUFS = 2
FCO_BUFS = 2


def _wrap_idx(flat_idx: np.ndarray) -> np.ndarray:
    """Wrap a flat int16 index array [n] (n % 16 == 0) into the dma_gather
    SBUF layout [128, n//16]: position j -> (partition j%16, column j//16),
    replicated across the eight 16-partition bands."""
    n = flat_idx.shape[0]
    band = flat_idx.reshape(n // 16, 16).T  # [16, n//16]
    return np.tile(band, (8, 1)).astype(np.int16)


def _split_subs(l: int) -> list[tuple[int, int]]:
    """Split l columns into (offset, size) sub-pieces of size <= LMAX."""
    subs = []
    o = 0
    while o < l:
        sz = min(LMAX, l - o)
        subs.append((o, sz))
        o += sz
    return subs


def _preprocess(x, edge_index, W1, b1, W2, b2, Wfc, bfc):
    """All host-side graph preprocessing. Returns (plan, in_maps, ids_order)."""
    row = np.asarray(edge_index[0], dtype=np.int64)
    col = np.asarray(edge_index[1], dtype=np.int64)

    deg = np.bincount(col, minlength=N_NODES).astype(np.int64) + 1
    dinv = (1.0 / np.sqrt(deg.astype(np.float32))).astype(np.float32)

    # --- node -> (core, slot) assignment, built to minimize gather padding.
    # A node's "chunk" as a message SOURCE is core//2 (4 chunks of 2 cores,
    # 25088 table rows each; dma_gather int16 indices only reach 32767 rows,
    # hence the chunked gather).  We greedily color nodes into the 4 chunks
    # so that every dest's in-edges are spread evenly over chunks; then the
    # per-(block,chunk) max padding is small.  Within a chunk, dests are
    # sorted by their count vector and striped across the chunk's 2 cores.
    rng = np.random.default_rng(12345)
    all_src0 = np.concatenate([row, np.arange(N_NODES)])
    all_dst0 = np.concatenate([col, np.arange(N_NODES)])
    o = np.argsort(all_src0, kind="stable")
    sr = all_src0[o]
    sc = all_dst0[o]
    starts = np.searchsorted(sr, np.arange(N_NODES + 1))
    deg_out = np.diff(starts)
    target = deg.astype(np.float32) / NCHUNK

    CAP = CH_ROWS - 64  # leave pad rows in every chunk
    color = np.full(N_NODES, -1, np.int8)
    kmat = np.zeros((N_NODES, NCHUNK), np.int32)
    sizes = np.zeros(NCHUNK, np.int64)
    order_src = rng.permutation(N_NODES)
    B = 1000
    for i in range(0, N_NODES, B):
        batch = order_src[i:i + B]
        reps = deg_out[batch]
        idx = np.concatenate(
            [np.arange(starts[s], starts[s + 1]) for s in batch]
        )
        dsts = sc[idx]
        srcrep = np.repeat(np.arange(len(batch)), reps)
        dev = kmat[dsts].astype(np.float32) - target[dsts][:, None]
        score = np.zeros((len(batch), NCHUNK), np.float32)
        np.add.at(score, srcrep, dev)
        score += (sizes / CAP).astype(np.float32) * 0.5 * reps[:, None]
        score[:, sizes >= CAP] = 1e18
        ch = score.argmin(1).astype(np.int8)
        color[batch] = ch
        np.add.at(sizes, ch, 1)
        np.add.at(kmat, (dsts, ch[srcrep]), 1)

    # refinement passes: re-place each source greedily, penalizing pushing a
    # dest above its balanced per-chunk ceiling (reduces gather padding)
    BR = 256
    MEAN = N_NODES / NCHUNK
    for _ in range(3):
        order = rng.permutation(N_NODES)
        for i in range(0, N_NODES, BR):
            batch = order[i:i + BR]
            reps = deg_out[batch]
            idx = np.concatenate(
                [np.arange(starts[s], starts[s + 1]) for s in batch]
            )
            dsts = sc[idx]
            srcrep = np.repeat(np.arange(len(batch)), reps)
            cur = color[batch]
            np.add.at(kmat, (dsts, cur[srcrep]), -1)
            np.add.at(sizes, cur, -1)
            dev = kmat[dsts].astype(np.float32) - target[dsts][:, None]
            ceil_bal = (deg[dsts] + NCHUNK - 1) // NCHUNK
            over = (kmat[dsts] + 1 > ceil_bal[:, None]).astype(np.float32) * 10.0
            score = np.zeros((len(batch), NCHUNK), np.float32)
            np.add.at(score, srcrep, dev + over)
            score += ((sizes - MEAN) * 0.15).astype(np.float32) * \
                np.maximum(reps, 1)[:, None] / 33.0
            ch = score.argmin(1).astype(np.int8)
            color[batch] = ch
            np.add.at(sizes, ch, 1)
            np.add.at(kmat, (dsts, ch[srcrep]), 1)
    assert sizes.max() <= CH_ROWS - 2, sizes

    slot = np.empty(N_NODES, np.int64)
    core_of = np.empty(N_NODES, np.int64)
    ids_order = []
    n_core = [0] * N_CORES
    for q in range(NCHUNK):
        nodes_q = np.where(color == q)[0]
        kk = kmat[nodes_q]
        mm = kk.max(1)
        o2 = np.lexsort((-kk[:, 3], -kk[:, 2], -kk[:, 1], -kk[:, 0], -mm))
        nq = nodes_q[o2]
        r = np.arange(len(nq))
        core_of[nq] = 2 * q + (r % 2)
        slot[nq] = (r // 256) * P + (r % 256) // 2
    pos = core_of * SLOTS + slot
    for c in range(N_CORES):
        ids = np.where(core_of == c)[0]
        ids = ids[np.argsort(slot[ids])]
        ids_order.append(ids)
        n_core[c] = len(ids)
        assert n_core[c] <= ZERO_LOCAL
        assert np.array_equal(slot[ids], np.arange(len(ids)))

    # --- edge lists sorted by (dest position, src chunk); self loops included
    all_src = np.concatenate([row, np.arange(N_NODES)])
    all_dst = np.concatenate([col, np.arange(N_NODES)])
    dst_pos = pos[all_dst]
    src_pos = pos[all_src]
    s_chunk = src_pos // CH_ROWS
    order = np.lexsort((s_chunk, dst_pos))
    dst_pos = dst_pos[order]
    src_pos = src_pos[order]
    s_chunk = s_chunk[order]

    # per (dest position, chunk) counts and CSR starts
    key = dst_pos * NCHUNK + s_chunk
    kcnt = np.bincount(key, minlength=N_CORES * SLOTS * NCHUNK).reshape(
        N_CORES, SLOTS, NCHUNK
    )
    csr = np.zeros(N_CORES * SLOTS * NCHUNK + 1, np.int64)
    np.cumsum(kcnt.ravel(), out=csr[1:])

    # --- groups of blocks; shared per-(group,chunk) column count l
    groups = []
    b0 = 0
    while b0 < BLOCKS:
        g = min(GMAX, BLOCKS - b0)
        groups.append((b0, g))
        b0 += g

    kblk = kcnt.reshape(N_CORES, BLOCKS, P, NCHUNK)
    ltab = np.zeros((len(groups), NCHUNK), np.int32)
    for gi, (b0, g) in enumerate(groups):
        for q in range(NCHUNK):
            ltab[gi, q] = max(1, int(kblk[:, b0:b0 + g, :, q].max()))

    real_edges = int(kcnt.sum())
    padded_cols = int(sum(ltab[gi, q] * g for gi, (b0, g) in enumerate(groups)
                          for q in range(NCHUNK)))
    padded_edges = padded_cols * P
    plan_inflation = padded_edges * 1.0 / real_edges

    # --- per-core index arrays + compile-time call plan
    # Plan entry per (group, chunk, sub): (q, sz, piece_col0, idx_off16,
    # ncalls).  idx arrays are ordered (group, chunk, sub, call) and each
    # call is exactly 1024 idx wrapped to [128, 64] int16.
    p_ar = np.arange(P)
    plan_groups = []   # [ (b0, g, idx_off16, idx_len16, [ (q, subs=[(o,sz,piece_col0)...]) ] ) ]
    idx_arrays = [[] for _ in range(N_CORES)]
    off16 = 0
    for gi, (b0, g) in enumerate(groups):
        g_off16 = off16
        qplans = []
        for q in range(NCHUNK):
            l = int(ltab[gi, q])
            subs = []
            piece_col0 = 0
            for (o, sz) in _split_subs(l):
                subs.append((o, sz, piece_col0))
                piece_col0 += g * sz
                ncols = g * sz
                off16 += P * (ncols * P // 16)
            qplans.append((q, l, subs))
        # build idx values for every core for this (group): shape per q:
        # [g, l, P] -> subs -> calls.  The group's calls are laid side by
        # side ([128, W] tile, row-major in DRAM) so ONE dma_start loads
        # the whole group's indices.
        for c in range(N_CORES):
            glist = []
            for (q, l, subs) in qplans:
                slots_g = c * SLOTS + (b0 + np.arange(g))[:, None] * P + p_ar[None, :]
                base = csr[slots_g * NCHUNK + q]          # [g, P]
                kreal = kblk[c, b0:b0 + g, :, q]          # [g, P]
                jj = np.arange(l)
                valid = jj[None, :, None] < kreal[:, None, :]          # [g, l, P]
                src_take = np.minimum(jj[None, :, None], kreal[:, None, :] - 1)
                rowsel = src_pos[base[:, None, :] + src_take]          # [g, l, P]
                zero_row = q * CH_ROWS + ZERO_LOCAL
                gather_rows = np.where(valid, rowsel, zero_row)
                local = (gather_rows - q * CH_ROWS).astype(np.int16)   # [g, l, P]
                for (o, sz, pc0) in subs:
                    slab = local[:, o:o + sz, :].reshape(g * sz, P)    # cols-major
                    ncols = g * sz
                    for k0 in range(0, ncols, CALL_COLS):
                        callcols = slab[k0:min(k0 + CALL_COLS, ncols)]
                        glist.append(_wrap_idx(callcols.reshape(-1)))
            idx_arrays[c].append(np.concatenate(glist, axis=1))
        plan_groups.append((b0, g, g_off16, off16 - g_off16,
                            [(q, l, subs) for (q, l, subs) in qplans]))

    idx_in = [np.concatenate([a.reshape(-1) for a in idx_arrays[c]])
              for c in range(N_CORES)]
    assert idx_in[0].shape[0] == off16

    # --- per-core dense inputs
    xb = np.ascontiguousarray(x).astype(BF16)
    in_maps = []
    w1_in = np.ascontiguousarray(
        W1.astype(BF16).reshape(NCHUNK, P, HID).transpose(1, 0, 2).reshape(P, NCHUNK * HID)
    )
    w2_in = np.ascontiguousarray(
        W2.astype(BF16).reshape(2, P, HID).transpose(1, 0, 2).reshape(P, 2 * HID)
    )
    wfc_in = np.ascontiguousarray(
        Wfc.astype(BF16).reshape(2, P, NCLS).transpose(1, 0, 2).reshape(P, 2 * NCLS)
    )
    has_b1 = bool(np.any(b1)) ; has_b2 = bool(np.any(b2)) ; has_bfc = bool(np.any(bfc))
    b1_in = np.tile(np.asarray(b1, np.float32)[None, :], (P, 1))
    b2_in = np.tile(np.asarray(b2, np.float32)[None, :], (P, 1))
    bfc_in = np.tile(np.asarray(bfc, np.float32)[None, :], (P, 1))

    for c in range(N_CORES):
        A = np.zeros((SLOTS, IN_DIM), BF16)
        A[:n_core[c]] = xb[ids_order[c]]
        xtt = np.ascontiguousarray(
            A.reshape(BLOCKS, P, NCHUNK, P).transpose(0, 3, 2, 1).reshape(BLOCKS, P, IN_DIM)
        )
        dv = np.ones(SLOTS, np.float32)
        dv[:n_core[c]] = dinv[ids_order[c]]
        dvp = np.ascontiguousarray(dv.reshape(BLOCKS, P).T)  # [128, 98]
        m = {
            "xtt": xtt,
            "dinvp": dvp,
            "idxs": idx_in[c],
            "w1": w1_in,
            "w2": w2_in,
            "wfc": wfc_in,
        }
        if has_b1:
            m["b1b"] = b1_in
        if has_b2:
            m["b2b"] = b2_in
        if has_bfc:
            m["bfcb"] = bfc_in
        in_maps.append(m)

    plan = {
        "groups": plan_groups,
        "idx_total": off16,
        "has_b1": has_b1,
        "has_b2": has_b2,
        "has_bfc": has_bfc,
        "inflation": plan_inflation,
        "n_core": n_core,
    }
    return plan, in_maps, ids_order


def _build_program(plan, sim_single_core=False, stop_after="full", debug_dumps=False):
    """Build the SPMD Bass program (one program, all cores).

    stop_after: one of "mm1", "ag1", "g1", "ag2", "g2", "full" — truncates
    the program after that phase (for bisection/debug)."""
    STAGES = ["mm1", "ag1", "g1", "ag2", "g2", "full"]
    stop_idx = STAGES.index(stop_after)
    nc = bacc.Bacc("TRN2", target_bir_lowering=False, debug=False,
                   num_devices=N_CORES)
    dt = mybir.dt

    xtt = nc.dram_tensor("xtt", [BLOCKS, P, IN_DIM], dt.bfloat16, kind="ExternalInput")
    dinvp = nc.dram_tensor("dinvp", [P, BLOCKS], dt.float32, kind="ExternalInput")
    idxs = nc.dram_tensor("idxs", [plan["idx_total"]], dt.int16, kind="ExternalInput")
    w1 = nc.dram_tensor("w1", [P, NCHUNK * HID], dt.bfloat16, kind="ExternalInput")
    w2 = nc.dram_tensor("w2", [P, 2 * HID], dt.bfloat16, kind="ExternalInput")
    wfc = nc.dram_tensor("wfc", [P, 2 * NCLS], dt.bfloat16, kind="ExternalInput")
    b1b = (nc.dram_tensor("b1b", [P, HID], dt.float32, kind="ExternalInput")
           if plan["has_b1"] else None)
    b2b = (nc.dram_tensor("b2b", [P, HID], dt.float32, kind="ExternalInput")
           if plan["has_b2"] else None)
    bfcb = (nc.dram_tensor("bfcb", [P, NCLS], dt.float32, kind="ExternalInput")
            if plan["has_bfc"] else None)
    out = nc.dram_tensor("out", [SLOTS, NCLS], dt.bfloat16, kind="ExternalOutput")

    hloc1 = nc.dram_tensor("hloc1", [SLOTS, HID], dt.bfloat16)
    hloc2 = nc.dram_tensor("hloc2", [SLOTS, HID], dt.bfloat16)
    hfull1 = nc.dram_tensor("hfull1", [N_CORES * SLOTS, HID], dt.bfloat16,
                            addr_space="Shared")
    hfull2 = nc.dram_tensor("hfull2", [N_CORES * SLOTS, HID], dt.bfloat16,
                            addr_space="Shared")

    groups = plan["groups"]

    with tile.TileContext(nc) as tc:
        with (
            tc.tile_pool(name="const", bufs=1) as constp,
            tc.tile_pool(name="xt", bufs=XT_BUFS) as xtp,
            tc.tile_pool(name="hst", bufs=HST_BUFS) as hstp,
            tc.tile_pool(name="idx", bufs=IDX_BUFS) as idxp,
            tc.tile_pool(name="pc", bufs=PC_BUFS) as pcp,
            tc.tile_pool(name="acc", bufs=ACC_BUFS) as accp,
            tc.tile_pool(name="hfin", bufs=HFIN_BUFS) as hfinp,
            tc.tile_pool(name="ht", bufs=HT_BUFS) as htp,
            tc.tile_pool(name="fco", bufs=FCO_BUFS) as fcop,
            tc.tile_pool(name="mmps", bufs=2, space="PSUM") as mmps,
            tc.tile_pool(name="tpps", bufs=2, space="PSUM") as tpps,
            tc.tile_pool(name="fcps", bufs=2, space="PSUM") as fcps,
        ):
            # resident constants
            w1_sb = constp.tile([P, NCHUNK * HID], dt.bfloat16)
            nc.sync.dma_start(out=w1_sb[:], in_=w1[:])
            w2_sb = constp.tile([P, 2 * HID], dt.bfloat16)
            nc.sync.dma_start(out=w2_sb[:], in_=w2[:])
            wfc_sb = constp.tile([P, 2 * NCLS], dt.bfloat16)
            nc.sync.dma_start(out=wfc_sb[:], in_=wfc[:])
            dv_sb = constp.tile([P, BLOCKS], dt.float32)
            nc.sync.dma_start(out=dv_sb[:], in_=dinvp[:])
            ident = constp.tile([P, P], dt.bfloat16)
            make_identity(nc, ident[:])
            b1_sb = b2_sb = bfc_sb = None
            if b1b is not None:
                b1_sb = constp.tile([P, HID], dt.float32)
                nc.sync.dma_start(out=b1_sb[:], in_=b1b[:])
            if b2b is not None:
                b2_sb = constp.tile([P, HID], dt.float32)
                nc.sync.dma_start(out=b2_sb[:], in_=b2b[:])
            if bfcb is not None:
                bfc_sb = constp.tile([P, NCLS], dt.float32)
                nc.sync.dma_start(out=bfc_sb[:], in_=bfcb[:])

            def layer1_matmul():
                """hloc1[b] = dinv * (x @ W1) as bf16, group-batched DMA."""
                for (b0, g, _, _, _) in groups:
                    at = xtp.tile([P, g * IN_DIM], dt.bfloat16, tag="xt")
                    nc.sync.dma_start(
                        out=at[:].rearrange("p (g f) -> p g f", g=g),
                        in_=xtt[b0:b0 + g].rearrange("g p f -> p g f"),
                    )
                    hs = hstp.tile([P, g * HID], dt.bfloat16, tag="hs")
                    for bl in range(g):
                        ps = mmps.tile([P, HID], dt.float32, space="PSUM", tag="mm")
                        for k in range(NCHUNK):
                            nc.tensor.matmul(
                                out=ps[:],
                                lhsT=at[:, bl * IN_DIM + k * P: bl * IN_DIM + (k + 1) * P],
                                rhs=w1_sb[:, k * HID:(k + 1) * HID],
                                start=(k == 0),
                                stop=(k == NCHUNK - 1),
                            )
                        nc.scalar.activation(
                            out=hs[:, bl * HID:(bl + 1) * HID], in_=ps[:],
                            func=mybir.ActivationFunctionType.Copy,
                            scale=dv_sb[:, b0 + bl:b0 + bl + 1],
                        )
                    nc.sync.dma_start(
                        out=hloc1[b0 * P:(b0 + g) * P, :].rearrange(
                            "(g p) f -> p g f", p=P),
                        in_=hs[:].rearrange("p (g f) -> p g f", g=g),
                    )

            def all_gather(hloc, hfull):
                if sim_single_core:
                    nc.sync.dma_start(out=hfull[0:SLOTS, :], in_=hloc[:])
                else:
                    nc.gpsimd.collective_compute(
                        "AllGather",
                        mybir.AluOpType.bypass,
                        replica_groups=[list(range(N_CORES))],
                        ins=[hloc[:]],
                        outs=[hfull[:]],
                    )

            def gather_layer(hfull, b_sb, last):
                """Per group: gather + segmented-sum + relu; then fused
                transpose + next-layer matmul per block.
                last=False: produce hloc2 (layer-2 input);
                last=True:  produce FC output rows (bf16)."""
                for (b0, g, g_off16, g_len16, qplans) in groups:
                    it = idxp.tile([P, g_len16 // P], dt.int16, tag="idx")
                    nc.sync.dma_start(
                        out=it[:],
                        in_=idxs[g_off16:g_off16 + g_len16].rearrange(
                            "(p s) -> p s", p=P),
                    )
                    acc = accp.tile([P, g * HID], dt.bfloat16, tag="acc")
                    first = True
                    ioff = 0  # int16 columns consumed within `it`
                    for (q, l, subs) in qplans:
                        for (o, sz, pc0) in subs:
                            ncols = g * sz
                            pc = pcp.tile([P, ncols * HID], dt.bfloat16, tag="pc")
                            for k0 in range(0, ncols, CALL_COLS):
                                kc = min(CALL_COLS, ncols - k0)
                                nidx = kc * P
                                nc.gpsimd.dma_gather(
                                    pc[:, k0 * HID:(k0 + kc) * HID]
                                    .rearrange("p (l d) -> p l d", d=HID),
                                    hfull[q * CH_ROWS:(q + 1) * CH_ROWS, :],
                                    it[:, ioff:ioff + kc * CALL_COLS],
                                    nidx,
                                    nidx,
                                    HID,
                                )
                                ioff += kc * CALL_COLS
                            # fold-tree over sz columns within each block:
                            # view [128, g, sz*HID], halve along sz
                            pcv = pc[:].rearrange("p (b x) -> p b x", b=g)
                            cur = sz
                            while cur > 1:
                                half = cur // 2
                                keep = cur - half
                                nc.vector.tensor_tensor(
                                    out=pcv[:, :, 0:half * HID],
                                    in0=pcv[:, :, 0:half * HID],
                                    in1=pcv[:, :, keep * HID:(keep + half) * HID],
                                    op=mybir.AluOpType.add,
                                )
                                cur = keep
                            accv = acc[:].rearrange("p (b x) -> p b x", b=g)
                            if first:
                                nc.vector.tensor_copy(
                                    out=accv[:, :, 0:HID], in_=pcv[:, :, 0:HID]
                                )
                                first = False
                            else:
                                nc.vector.tensor_tensor(
                                    out=accv[:, :, 0:HID],
                                    in0=accv[:, :, 0:HID],
                                    in1=pcv[:, :, 0:HID],
                                    op=mybir.AluOpType.add,
                                )
                    # finalize + fused transpose / next-layer matmul
                    hf = hfinp.tile([P, g * HID], dt.bfloat16, tag="hf")
                    if last:
                        ho = fcop.tile([P, g * NCLS], dt.bfloat16, tag="fco", name="ho_fc")
                    else:
                        ho = hstp.tile([P, g * HID], dt.bfloat16, tag="hs2", name="ho_h2")
                    for bl in range(g):
                        if b_sb is not None:
                            hff = hfinp.tile([P, HID], dt.float32, tag="hff")
                            nc.vector.tensor_scalar(
                                out=hff[:], in0=acc[:, bl * HID:(bl + 1) * HID],
                                scalar1=dv_sb[:, b0 + bl:b0 + bl + 1], scalar2=None,
                                op0=mybir.AluOpType.mult,
                            )
                            nc.vector.tensor_tensor(
                                out=hff[:], in0=hff[:], in1=b_sb[:],
                                op=mybir.AluOpType.add,
                            )
                            nc.scalar.activation(
                                out=hf[:, bl * HID:(bl + 1) * HID], in_=hff[:],
                                func=mybir.ActivationFunctionType.Relu,
                            )
                        else:
                            nc.scalar.activation(
                                out=hf[:, bl * HID:(bl + 1) * HID],
                                in_=acc[:, bl * HID:(bl + 1) * HID],
                                func=mybir.ActivationFunctionType.Relu,
                                scale=dv_sb[:, b0 + bl:b0 + bl + 1],
                            )
                        # transpose h block -> lhsT chunks [feat 128, nodes 128]
                        ht = htp.tile([P, HID], dt.bfloat16, tag="ht")
                        for k in range(2):
                            tp = tpps.tile([P, P], dt.bfloat16, space="PSUM", tag="tp")
                            nc.tensor.transpose(
                                out=tp[:],
                                in_=hf[:, bl * HID + k * P: bl * HID + (k + 1) * P],
                                identity=ident[:],
                            )
                            nc.scalar.copy(out=ht[:, k * P:(k + 1) * P], in_=tp[:])
                        if not last:
                            ps = mmps.tile([P, HID], dt.float32, space="PSUM", tag="mm")
                            for k in range(2):
                                nc.tensor.matmul(
                                    out=ps[:],
                                    lhsT=ht[:, k * P:(k + 1) * P],
                                    rhs=w2_sb[:, k * HID:(k + 1) * HID],
                                    start=(k == 0),
                                    stop=(k == 1),
                                )
                            nc.scalar.activation(
                                out=ho[:, bl * HID:(bl + 1) * HID], in_=ps[:],
                                func=mybir.ActivationFunctionType.Copy,
                                scale=dv_sb[:, b0 + bl:b0 + bl + 1],
                            )
                        else:
                            for n in range(2):
                                ps = fcps.tile([P, NCLS // 2], dt.float32,
                                               space="PSUM", tag="fc")
                                for k in range(2):
                                    nc.tensor.matmul(
                                        out=ps[:],
                                        lhsT=ht[:, k * P:(k + 1) * P],
                                        rhs=wfc_sb[:, k * NCLS + n * (NCLS // 2):
                                                   k * NCLS + (n + 1) * (NCLS // 2)],
                                        start=(k == 0),
                                        stop=(k == 1),
                                    )
                                if bfc_sb is not None:
                                    fo32 = hfinp.tile([P, NCLS // 2], dt.float32,
                                                      tag="fo32")
                                    nc.vector.tensor_tensor(
                                        out=fo32[:], in0=ps[:],
                                        in1=bfc_sb[:, n * (NCLS // 2):(n + 1) * (NCLS // 2)],
                                        op=mybir.AluOpType.add,
                                    )
                                    nc.vector.tensor_copy(
                                        out=ho[:, bl * NCLS + n * (NCLS // 2):
                                               bl * NCLS + (n + 1) * (NCLS // 2)],
                                        in_=fo32[:],
                                    )
                                else:
                                    nc.vector.tensor_copy(
                                        out=ho[:, bl * NCLS + n * (NCLS // 2):
                                               bl * NCLS + (n + 1) * (NCLS // 2)],
                                        in_=ps[:],
                                    )
                    if not last:
                        nc.sync.dma_start(
                            out=hloc2[b0 * P:(b0 + g) * P, :].rearrange(
                                "(g p) f -> p g f", p=P),
                            in_=ho[:].rearrange("p (g f) -> p g f", g=g),
                        )
                    else:
                        nc.sync.dma_start(
                            out=out[b0 * P:(b0 + g) * P, :].rearrange(
                                "(g p) f -> p g f", p=P),
                            in_=ho[:].rearrange("p (g f) -> p g f", g=g),
                        )

            # ---- layer 1
            layer1_matmul()
            if stop_idx >= 1:
                all_gather(hloc1, hfull1)
            if stop_idx >= 2:
                gather_layer(hfull1, b1_sb, last=False)
            # ---- layer 2
            if stop_idx >= 3:
                all_gather(hloc2, hfull2)
            if stop_idx >= 4:
                gather_layer(hfull2, b2_sb, last=True)
            if debug_dumps:
                hl1d = nc.dram_tensor("hl1d", [SLOTS, HID], dt.bfloat16,
                                      kind="ExternalOutput")
                hl2d = nc.dram_tensor("hl2d", [SLOTS, HID], dt.bfloat16,
                                      kind="ExternalOutput")
                hf1d = nc.dram_tensor("hf1d", [N_CORES * SLOTS, HID], dt.bfloat16,
                                      kind="ExternalOutput")
                nc.sync.dma_start(out=hl1d[:], in_=hloc1[:])
                nc.sync.dma_start(out=hl2d[:], in_=hloc2[:])
                nc.sync.dma_start(out=hf1d[:], in_=hfull1[:])

    nc.compile()
    return nc


def kernel(x, edge_index, W1, b1, W2, b2, Wfc, bfc):
    x = np.asarray(x, dtype=np.float32)
    edge_index = np.asarray(edge_index)
    W1 = np.asarray(W1, dtype=np.float32)
    b1 = np.asarray(b1, dtype=np.float32)
    W2 = np.asarray(W2, dtype=np.float32)
    b2 = np.asarray(b2, dtype=np.float32)
    Wfc = np.asarray(Wfc, dtype=np.float32)
    bfc = np.asarray(bfc, dtype=np.float32)
    plan, in_maps, ids_order = _preprocess(x, edge_index, W1, b1, W2, b2, Wfc, bfc)
    nc = _build_program(plan)
    res = run_bass_kernel_spmd(nc, in_maps, core_ids=list(range(N_CORES)))
    full = np.empty((N_NODES, NCLS), np.float32)
    for c in range(N_CORES):
        full[ids_order[c]] = res.results[c]["out"][: len(ids_order[c])].astype(np.float32)
    return full


# revision 24
# speedup vs baseline: 1.0079x; 1.0079x over previous
"""Trainium2 Bass kernel for a 2-layer GCN + FC head (nn_CNNGNNModel).

Reference computation (PyG GCNConv semantics, symmetric normalization with
self-loops):
    deg[i]  = in-degree(i) + 1 ;  dinv = deg^-0.5
    A_hat   = D^-1/2 (A + I) D^-1/2   (aggregation by destination)
    h1 = relu(A_hat @ (x @ W1) + b1)
    h2 = relu(A_hat @ (h1 @ W2) + b2)
    out = h2 @ Wfc + bfc

The per-edge weight dinv[src]*dinv[dst] is separable: node features are
scaled by dinv on the way out of each matmul (source side) and the
aggregate is scaled by dinv after the segmented sum (dest side), so message
passing is a pure gather + segmented sum.

Distribution (8 NeuronCores, SPMD single program):
  - Nodes are sharded by id range: core c owns dests [c*12500, (c+1)*12500),
    padded to 12544 = 98*128 slots.  All index plumbing is precomputed on
    the host in "position" space pos = owner*12544 + slot.
  - Each layer: local matmul of the core's node block -> dinv-scaled bf16
    features -> AllGather to a full table [100352, 256] bf16 -> dma_gather
    of incoming messages -> DVE segmented sum -> relu -> (fused) transpose
    + next-layer matmul per block.
  - dma_gather indices are int16 (<= 32767); the table is addressed in 4
    residue "chunks" of 25088 rows via the in_ AP base offset.  The HW
    SWDGE limit is 1024 indices per call, so all calls are packed to
    exactly 1024 indices (8 columns x 128 dests): blocks are processed in
    groups of 4, and per (group, chunk) the per-dest message counts are
    padded to a shared even column count l, giving uniform [128, 4, l, 256]
    tiles whose segmented sum is a log2(l) fold-tree of wide strided DVE
    adds.
  - Weights are replicated; output [12544, 1000] bf16 per core is cast to
    f32 and reassembled (inverse permutation) on the host.
"""

import numpy as np
import ml_dtypes

import concourse.bass as bass
import concourse.bacc as bacc
import concourse.mybir as mybir
import concourse.tile as tile
from concourse.bass_utils import run_bass_kernel_spmd
from concourse.masks import make_identity

BF16 = ml_dtypes.bfloat16

N_CORES = 8
N_NODES = 100000
IN_DIM = 512
HID = 256
NCLS = 1000
NLOC = 12500          # real dests per core
SLOTS = 12544         # padded dests per core (98 blocks of 128)
BLOCKS = SLOTS // 128  # 98
NCHUNK = 4
CH_ROWS = 2 * SLOTS   # 25088 rows per chunk (= 2 cores)
ZERO_LOCAL = 12543    # chunk-local row guaranteed to be a zero pad row
P = 128
GMAX = 4              # blocks per group
LMAX = 12             # max fold columns per sub-piece (even)
CALL_COLS = 8         # 8 cols * 128 = 1024 idx per dma_gather (HW limit)
# tile-pool depths (pipelining knobs)
XT_BUFS = 2
HST_BUFS = 2
IDX_BUFS = 3
PC_BUFS = 5
ACC_BUFS = 3
HFIN_BUFS = 2
HT_BUFS = 2
FCO_BUFS = 2


def _wrap_idx(flat_idx: np.ndarray) -> np.ndarray:
    """Wrap a flat int16 index array [n] (n % 16 == 0) into the dma_gather
    SBUF layout [128, n//16]: position j -> (partition j%16, column j//16),
    replicated across the eight 16-partition bands."""
    n = flat_idx.shape[0]
    band = flat_idx.reshape(n // 16, 16).T  # [16, n//16]
    return np.tile(band, (8, 1)).astype(np.int16)


def _split_subs(l: int) -> list[tuple[int, int]]:
    """Split l columns into (offset, size) sub-pieces of size <= LMAX."""
    subs = []
    o = 0
    while o < l:
        sz = min(LMAX, l - o)
        subs.append((o, sz))
        o += sz
    return subs


def _preprocess(x, edge_index, W1, b1, W2, b2, Wfc, bfc):
    """All host-side graph preprocessing. Returns (plan, in_maps, ids_order)."""
    row = np.asarray(edge_index[0], dtype=np.int64)
    col = np.asarray(edge_index[1], dtype=np.int64)

    deg = np.bincount(col, minlength=N_NODES).astype(np.int64) + 1
    dinv = (1.0 / np.sqrt(deg.astype(np.float32))).astype(np.float32)

    # --- node -> (core, slot) assignment, built to minimize gather padding.
    # A node's "chunk" as a message SOURCE is core//2 (4 chunks of 2 cores,
    # 25088 table rows each; dma_gather int16 indices only reach 32767 rows,
    # hence the chunked gather).  We greedily color nodes into the 4 chunks
    # so that every dest's in-edges are spread evenly over chunks; then the
    # per-(block,chunk) max padding is small.  Within a chunk, dests are
    # sorted by their count vector and striped across the chunk's 2 cores.
    rng = np.random.default_rng(12345)
    all_src0 = np.concatenate([row, np.arange(N_NODES)])
    all_dst0 = np.concatenate([col, np.arange(N_NODES)])
    o = np.argsort(all_src0, kind="stable")
    sr = all_src0[o]
    sc = all_dst0[o]
    starts = np.searchsorted(sr, np.arange(N_NODES + 1))
    deg_out = np.diff(starts)
    target = deg.astype(np.float32) / NCHUNK

    CAP = CH_ROWS - 64  # leave pad rows in every chunk
    color = np.full(N_NODES, -1, np.int8)
    kmat = np.zeros((N_NODES, NCHUNK), np.int32)
    sizes = np.zeros(NCHUNK, np.int64)
    order_src = rng.permutation(N_NODES)
    B = 1000
    for i in range(0, N_NODES, B):
        batch = order_src[i:i + B]
        reps = deg_out[batch]
        idx = np.concatenate(
            [np.arange(starts[s], starts[s + 1]) for s in batch]
        )
        dsts = sc[idx]
        srcrep = np.repeat(np.arange(len(batch)), reps)
        dev = kmat[dsts].astype(np.float32) - target[dsts][:, None]
        score = np.zeros((len(batch), NCHUNK), np.float32)
        np.add.at(score, srcrep, dev)
        score += (sizes / CAP).astype(np.float32) * 0.5 * reps[:, None]
        score[:, sizes >= CAP] = 1e18
        ch = score.argmin(1).astype(np.int8)
        color[batch] = ch
        np.add.at(sizes, ch, 1)
        np.add.at(kmat, (dsts, ch[srcrep]), 1)

    # refinement passes: re-place each source greedily, penalizing pushing a
    # dest above its balanced per-chunk ceiling (reduces gather padding)
    BR = 256
    MEAN = N_NODES / NCHUNK
    for _ in range(5):
        order = rng.permutation(N_NODES)
        for i in range(0, N_NODES, BR):
            batch = order[i:i + BR]
            reps = deg_out[batch]
            idx = np.concatenate(
                [np.arange(starts[s], starts[s + 1]) for s in batch]
            )
            dsts = sc[idx]
            srcrep = np.repeat(np.arange(len(batch)), reps)
            cur = color[batch]
            np.add.at(kmat, (dsts, cur[srcrep]), -1)
            np.add.at(sizes, cur, -1)
            dev = kmat[dsts].astype(np.float32) - target[dsts][:, None]
            ceil_bal = (deg[dsts] + NCHUNK - 1) // NCHUNK
            over = (kmat[dsts] + 1 > ceil_bal[:, None]).astype(np.float32) * 10.0
            score = np.zeros((len(batch), NCHUNK), np.float32)
            np.add.at(score, srcrep, dev + over)
            score += ((sizes - MEAN) * 0.15).astype(np.float32) * \
                np.maximum(reps, 1)[:, None] / 33.0
            ch = score.argmin(1).astype(np.int8)
            color[batch] = ch
            np.add.at(sizes, ch, 1)
            np.add.at(kmat, (dsts, ch[srcrep]), 1)
    assert sizes.max() <= CH_ROWS - 2, sizes

    slot = np.empty(N_NODES, np.int64)
    core_of = np.empty(N_NODES, np.int64)
    ids_order = []
    n_core = [0] * N_CORES
    for q in range(NCHUNK):
        nodes_q = np.where(color == q)[0]
        kk = kmat[nodes_q]
        mm = kk.max(1)
        o2 = np.lexsort((-kk[:, 3], -kk[:, 2], -kk[:, 1], -kk[:, 0], -mm))
        nq = nodes_q[o2]
        r = np.arange(len(nq))
        core_of[nq] = 2 * q + (r % 2)
        slot[nq] = (r // 256) * P + (r % 256) // 2
    pos = core_of * SLOTS + slot
    for c in range(N_CORES):
        ids = np.where(core_of == c)[0]
        ids = ids[np.argsort(slot[ids])]
        ids_order.append(ids)
        n_core[c] = len(ids)
        assert n_core[c] <= ZERO_LOCAL
        assert np.array_equal(slot[ids], np.arange(len(ids)))

    # --- edge lists sorted by (dest position, src chunk); self loops included
    all_src = np.concatenate([row, np.arange(N_NODES)])
    all_dst = np.concatenate([col, np.arange(N_NODES)])
    dst_pos = pos[all_dst]
    src_pos = pos[all_src]
    s_chunk = src_pos // CH_ROWS
    order = np.lexsort((s_chunk, dst_pos))
    dst_pos = dst_pos[order]
    src_pos = src_pos[order]
    s_chunk = s_chunk[order]

    # per (dest position, chunk) counts and CSR starts
    key = dst_pos * NCHUNK + s_chunk
    kcnt = np.bincount(key, minlength=N_CORES * SLOTS * NCHUNK).reshape(
        N_CORES, SLOTS, NCHUNK
    )
    csr = np.zeros(N_CORES * SLOTS * NCHUNK + 1, np.int64)
    np.cumsum(kcnt.ravel(), out=csr[1:])

    # --- groups of blocks; shared per-(group,chunk) column count l
    groups = []
    b0 = 0
    while b0 < BLOCKS:
        g = min(GMAX, BLOCKS - b0)
        groups.append((b0, g))
        b0 += g

    kblk = kcnt.reshape(N_CORES, BLOCKS, P, NCHUNK)
    ltab = np.zeros((len(groups), NCHUNK), np.int32)
    for gi, (b0, g) in enumerate(groups):
        for q in range(NCHUNK):
            ltab[gi, q] = max(1, int(kblk[:, b0:b0 + g, :, q].max()))

    real_edges = int(kcnt.sum())
    padded_cols = int(sum(ltab[gi, q] * g for gi, (b0, g) in enumerate(groups)
                          for q in range(NCHUNK)))
    padded_edges = padded_cols * P
    plan_inflation = padded_edges * 1.0 / real_edges

    # --- per-core index arrays + compile-time call plan
    # Plan entry per (group, chunk, sub): (q, sz, piece_col0, idx_off16,
    # ncalls).  idx arrays are ordered (group, chunk, sub, call) and each
    # call is exactly 1024 idx wrapped to [128, 64] int16.
    p_ar = np.arange(P)
    plan_groups = []   # [ (b0, g, idx_off16, idx_len16, [ (q, subs=[(o,sz,piece_col0)...]) ] ) ]
    idx_arrays = [[] for _ in range(N_CORES)]
    off16 = 0
    for gi, (b0, g) in enumerate(groups):
        g_off16 = off16
        qplans = []
        for q in range(NCHUNK):
            l = int(ltab[gi, q])
            subs = []
            piece_col0 = 0
            for (o, sz) in _split_subs(l):
                subs.append((o, sz, piece_col0))
                piece_col0 += g * sz
                ncols = g * sz
                off16 += P * (ncols * P // 16)
            qplans.append((q, l, subs))
        # build idx values for every core for this (group): shape per q:
        # [g, l, P] -> subs -> calls.  The group's calls are laid side by
        # side ([128, W] tile, row-major in DRAM) so ONE dma_start loads
        # the whole group's indices.
        for c in range(N_CORES):
            glist = []
            for (q, l, subs) in qplans:
                slots_g = c * SLOTS + (b0 + np.arange(g))[:, None] * P + p_ar[None, :]
                base = csr[slots_g * NCHUNK + q]          # [g, P]
                kreal = kblk[c, b0:b0 + g, :, q]          # [g, P]
                jj = np.arange(l)
                valid = jj[None, :, None] < kreal[:, None, :]          # [g, l, P]
                src_take = np.minimum(jj[None, :, None], kreal[:, None, :] - 1)
                rowsel = src_pos[base[:, None, :] + src_take]          # [g, l, P]
                zero_row = q * CH_ROWS + ZERO_LOCAL
                gather_rows = np.where(valid, rowsel, zero_row)
                local = (gather_rows - q * CH_ROWS).astype(np.int16)   # [g, l, P]
                for (o, sz, pc0) in subs:
                    slab = local[:, o:o + sz, :].reshape(g * sz, P)    # cols-major
                    ncols = g * sz
                    for k0 in range(0, ncols, CALL_COLS):
                        callcols = slab[k0:min(k0 + CALL_COLS, ncols)]
                        glist.append(_wrap_idx(callcols.reshape(-1)))
            idx_arrays[c].append(np.concatenate(glist, axis=1))
        plan_groups.append((b0, g, g_off16, off16 - g_off16,
                            [(q, l, subs) for (q, l, subs) in qplans]))

    idx_in = [np.concatenate([a.reshape(-1) for a in idx_arrays[c]])
              for c in range(N_CORES)]
    assert idx_in[0].shape[0] == off16

    # --- per-core dense inputs
    xb = np.ascontiguousarray(x).astype(BF16)
    in_maps = []
    w1_in = np.ascontiguousarray(
        W1.astype(BF16).reshape(NCHUNK, P, HID).transpose(1, 0, 2).reshape(P, NCHUNK * HID)
    )
    w2_in = np.ascontiguousarray(
        W2.astype(BF16).reshape(2, P, HID).transpose(1, 0, 2).reshape(P, 2 * HID)
    )
    wfc_in = np.ascontiguousarray(
        Wfc.astype(BF16).reshape(2, P, NCLS).transpose(1, 0, 2).reshape(P, 2 * NCLS)
    )
    has_b1 = bool(np.any(b1)) ; has_b2 = bool(np.any(b2)) ; has_bfc = bool(np.any(bfc))
    b1_in = np.tile(np.asarray(b1, np.float32)[None, :], (P, 1))
    b2_in = np.tile(np.asarray(b2, np.float32)[None, :], (P, 1))
    bfc_in = np.tile(np.asarray(bfc, np.float32)[None, :], (P, 1))

    for c in range(N_CORES):
        A = np.zeros((SLOTS, IN_DIM), BF16)
        A[:n_core[c]] = xb[ids_order[c]]
        xtt = np.ascontiguousarray(
            A.reshape(BLOCKS, P, NCHUNK, P).transpose(0, 3, 2, 1).reshape(BLOCKS, P, IN_DIM)
        )
        dv = np.ones(SLOTS, np.float32)
        dv[:n_core[c]] = dinv[ids_order[c]]
        dvp = np.ascontiguousarray(dv.reshape(BLOCKS, P).T)  # [128, 98]
        m = {
            "xtt": xtt,
            "dinvp": dvp,
            "idxs": idx_in[c],
            "w1": w1_in,
            "w2": w2_in,
            "wfc": wfc_in,
        }
        if has_b1:
            m["b1b"] = b1_in
        if has_b2:
            m["b2b"] = b2_in
        if has_bfc:
            m["bfcb"] = bfc_in
        in_maps.append(m)

    plan = {
        "groups": plan_groups,
        "idx_total": off16,
        "has_b1": has_b1,
        "has_b2": has_b2,
        "has_bfc": has_bfc,
        "inflation": plan_inflation,
        "n_core": n_core,
    }
    return plan, in_maps, ids_order


def _build_program(plan, sim_single_core=False, stop_after="full", debug_dumps=False):
    """Build the SPMD Bass program (one program, all cores).

    stop_after: one of "mm1", "ag1", "g1", "ag2", "g2", "full" — truncates
    the program after that phase (for bisection/debug)."""
    STAGES = ["mm1", "ag1", "g1", "ag2", "g2", "full"]
    stop_idx = STAGES.index(stop_after)
    nc = bacc.Bacc("TRN2", target_bir_lowering=False, debug=False,
                   num_devices=N_CORES)
    dt = mybir.dt

    xtt = nc.dram_tensor("xtt", [BLOCKS, P, IN_DIM], dt.bfloat16, kind="ExternalInput")
    dinvp = nc.dram_tensor("dinvp", [P, BLOCKS], dt.float32, kind="ExternalInput")
    idxs = nc.dram_tensor("idxs", [plan["idx_total"]], dt.int16, kind="ExternalInput")
    w1 = nc.dram_tensor("w1", [P, NCHUNK * HID], dt.bfloat16, kind="ExternalInput")
    w2 = nc.dram_tensor("w2", [P, 2 * HID], dt.bfloat16, kind="ExternalInput")
    wfc = nc.dram_tensor("wfc", [P, 2 * NCLS], dt.bfloat16, kind="ExternalInput")
    b1b = (nc.dram_tensor("b1b", [P, HID], dt.float32, kind="ExternalInput")
           if plan["has_b1"] else None)
    b2b = (nc.dram_tensor("b2b", [P, HID], dt.float32, kind="ExternalInput")
           if plan["has_b2"] else None)
    bfcb = (nc.dram_tensor("bfcb", [P, NCLS], dt.float32, kind="ExternalInput")
            if plan["has_bfc"] else None)
    out = nc.dram_tensor("out", [SLOTS, NCLS], dt.bfloat16, kind="ExternalOutput")

    hloc1 = nc.dram_tensor("hloc1", [SLOTS, HID], dt.bfloat16)
    hloc2 = nc.dram_tensor("hloc2", [SLOTS, HID], dt.bfloat16)
    hfull1 = nc.dram_tensor("hfull1", [N_CORES * SLOTS, HID], dt.bfloat16,
                            addr_space="Shared")
    hfull2 = nc.dram_tensor("hfull2", [N_CORES * SLOTS, HID], dt.bfloat16,
                            addr_space="Shared")

    groups = plan["groups"]

    with tile.TileContext(nc) as tc:
        with (
            tc.tile_pool(name="const", bufs=1) as constp,
            tc.tile_pool(name="xt", bufs=XT_BUFS) as xtp,
            tc.tile_pool(name="hst", bufs=HST_BUFS) as hstp,
            tc.tile_pool(name="idx", bufs=IDX_BUFS) as idxp,
            tc.tile_pool(name="pc", bufs=PC_BUFS) as pcp,
            tc.tile_pool(name="acc", bufs=ACC_BUFS) as accp,
            tc.tile_pool(name="hfin", bufs=HFIN_BUFS) as hfinp,
            tc.tile_pool(name="ht", bufs=HT_BUFS) as htp,
            tc.tile_pool(name="fco", bufs=FCO_BUFS) as fcop,
            tc.tile_pool(name="mmps", bufs=2, space="PSUM") as mmps,
            tc.tile_pool(name="tpps", bufs=2, space="PSUM") as tpps,
            tc.tile_pool(name="fcps", bufs=2, space="PSUM") as fcps,
        ):
            # resident constants
            w1_sb = constp.tile([P, NCHUNK * HID], dt.bfloat16)
            nc.sync.dma_start(out=w1_sb[:], in_=w1[:])
            w2_sb = constp.tile([P, 2 * HID], dt.bfloat16)
            nc.sync.dma_start(out=w2_sb[:], in_=w2[:])
            wfc_sb = constp.tile([P, 2 * NCLS], dt.bfloat16)
            nc.sync.dma_start(out=wfc_sb[:], in_=wfc[:])
            dv_sb = constp.tile([P, BLOCKS], dt.float32)
            nc.sync.dma_start(out=dv_sb[:], in_=dinvp[:])
            ident = constp.tile([P, P], dt.bfloat16)
            make_identity(nc, ident[:])
            b1_sb = b2_sb = bfc_sb = None
            if b1b is not None:
                b1_sb = constp.tile([P, HID], dt.float32)
                nc.sync.dma_start(out=b1_sb[:], in_=b1b[:])
            if b2b is not None:
                b2_sb = constp.tile([P, HID], dt.float32)
                nc.sync.dma_start(out=b2_sb[:], in_=b2b[:])
            if bfcb is not None:
                bfc_sb = constp.tile([P, NCLS], dt.float32)
                nc.sync.dma_start(out=bfc_sb[:], in_=bfcb[:])

            def layer1_matmul():
                """hloc1[b] = dinv * (x @ W1) as bf16, group-batched DMA."""
                for (b0, g, _, _, _) in groups:
                    at = xtp.tile([P, g * IN_DIM], dt.bfloat16, tag="xt")
                    nc.sync.dma_start(
                        out=at[:].rearrange("p (g f) -> p g f", g=g),
                        in_=xtt[b0:b0 + g].rearrange("g p f -> p g f"),
                    )
                    hs = hstp.tile([P, g * HID], dt.bfloat16, tag="hs")
                    for bl in range(g):
                        ps = mmps.tile([P, HID], dt.float32, space="PSUM", tag="mm")
                        for k in range(NCHUNK):
                            nc.tensor.matmul(
                                out=ps[:],
                                lhsT=at[:, bl * IN_DIM + k * P: bl * IN_DIM + (k + 1) * P],
                                rhs=w1_sb[:, k * HID:(k + 1) * HID],
                                start=(k == 0),
                                stop=(k == NCHUNK - 1),
                            )
                        nc.scalar.activation(
                            out=hs[:, bl * HID:(bl + 1) * HID], in_=ps[:],
                            func=mybir.ActivationFunctionType.Copy,
                            scale=dv_sb[:, b0 + bl:b0 + bl + 1],
                        )
                    nc.sync.dma_start(
                        out=hloc1[b0 * P:(b0 + g) * P, :].rearrange(
                            "(g p) f -> p g f", p=P),
                        in_=hs[:].rearrange("p (g f) -> p g f", g=g),
                    )

            def all_gather(hloc, hfull):
                if sim_single_core:
                    nc.sync.dma_start(out=hfull[0:SLOTS, :], in_=hloc[:])
                else:
                    nc.gpsimd.collective_compute(
                        "AllGather",
                        mybir.AluOpType.bypass,
                        replica_groups=[list(range(N_CORES))],
                        ins=[hloc[:]],
                        outs=[hfull[:]],
                    )

            def gather_layer(hfull, b_sb, last):
                """Per group: gather + segmented-sum + relu; then fused
                transpose + next-layer matmul per block.
                last=False: produce hloc2 (layer-2 input);
                last=True:  produce FC output rows (bf16)."""
                for (b0, g, g_off16, g_len16, qplans) in groups:
                    it = idxp.tile([P, g_len16 // P], dt.int16, tag="idx")
                    nc.sync.dma_start(
                        out=it[:],
                        in_=idxs[g_off16:g_off16 + g_len16].rearrange(
                            "(p s) -> p s", p=P),
                    )
                    acc = accp.tile([P, g * HID], dt.bfloat16, tag="acc")
                    first = True
                    ioff = 0  # int16 columns consumed within `it`
                    for (q, l, subs) in qplans:
                        for (o, sz, pc0) in subs:
                            ncols = g * sz
                            pc = pcp.tile([P, ncols * HID], dt.bfloat16, tag="pc")
                            for k0 in range(0, ncols, CALL_COLS):
                                kc = min(CALL_COLS, ncols - k0)
                                nidx = kc * P
                                nc.gpsimd.dma_gather(
                                    pc[:, k0 * HID:(k0 + kc) * HID]
                                    .rearrange("p (l d) -> p l d", d=HID),
                                    hfull[q * CH_ROWS:(q + 1) * CH_ROWS, :],
                                    it[:, ioff:ioff + kc * CALL_COLS],
                                    nidx,
                                    nidx,
                                    HID,
                                )
                                ioff += kc * CALL_COLS
                            # fold-tree over sz columns within each block:
                            # view [128, g, sz*HID], halve along sz
                            pcv = pc[:].rearrange("p (b x) -> p b x", b=g)
                            cur = sz
                            while cur > 1:
                                half = cur // 2
                                keep = cur - half
                                nc.vector.tensor_tensor(
                                    out=pcv[:, :, 0:half * HID],
                                    in0=pcv[:, :, 0:half * HID],
                                    in1=pcv[:, :, keep * HID:(keep + half) * HID],
                                    op=mybir.AluOpType.add,
                                )
                                cur = keep
                            accv = acc[:].rearrange("p (b x) -> p b x", b=g)
                            if first:
                                nc.vector.tensor_copy(
                                    out=accv[:, :, 0:HID], in_=pcv[:, :, 0:HID]
                                )
                                first = False
                            else:
                                nc.vector.tensor_tensor(
                                    out=accv[:, :, 0:HID],
                                    in0=accv[:, :, 0:HID],
                                    in1=pcv[:, :, 0:HID],
                                    op=mybir.AluOpType.add,
                                )
                    # finalize + fused transpose / next-layer matmul
                    hf = hfinp.tile([P, g * HID], dt.bfloat16, tag="hf")
                    if last:
                        ho = fcop.tile([P, g * NCLS], dt.bfloat16, tag="fco", name="ho_fc")
                    else:
                        ho = hstp.tile([P, g * HID], dt.bfloat16, tag="hs2", name="ho_h2")
                    for bl in range(g):
                        if b_sb is not None:
                            hff = hfinp.tile([P, HID], dt.float32, tag="hff")
                            nc.vector.tensor_scalar(
                                out=hff[:], in0=acc[:, bl * HID:(bl + 1) * HID],
                                scalar1=dv_sb[:, b0 + bl:b0 + bl + 1], scalar2=None,
                                op0=mybir.AluOpType.mult,
                            )
                            nc.vector.tensor_tensor(
                                out=hff[:], in0=hff[:], in1=b_sb[:],
                                op=mybir.AluOpType.add,
                            )
                            nc.scalar.activation(
                                out=hf[:, bl * HID:(bl + 1) * HID], in_=hff[:],
                                func=mybir.ActivationFunctionType.Relu,
                            )
                        else:
                            nc.scalar.activation(
                                out=hf[:, bl * HID:(bl + 1) * HID],
                                in_=acc[:, bl * HID:(bl + 1) * HID],
                                func=mybir.ActivationFunctionType.Relu,
                                scale=dv_sb[:, b0 + bl:b0 + bl + 1],
                            )
                        # transpose h block -> lhsT chunks [feat 128, nodes 128]
                        ht = htp.tile([P, HID], dt.bfloat16, tag="ht")
                        for k in range(2):
                            tp = tpps.tile([P, P], dt.bfloat16, space="PSUM", tag="tp")
                            nc.tensor.transpose(
                                out=tp[:],
                                in_=hf[:, bl * HID + k * P: bl * HID + (k + 1) * P],
                                identity=ident[:],
                            )
                            nc.scalar.copy(out=ht[:, k * P:(k + 1) * P], in_=tp[:])
                        if not last:
                            ps = mmps.tile([P, HID], dt.float32, space="PSUM", tag="mm")
                            for k in range(2):
                                nc.tensor.matmul(
                                    out=ps[:],
                                    lhsT=ht[:, k * P:(k + 1) * P],
                                    rhs=w2_sb[:, k * HID:(k + 1) * HID],
                                    start=(k == 0),
                                    stop=(k == 1),
                                )
                            nc.scalar.activation(
                                out=ho[:, bl * HID:(bl + 1) * HID], in_=ps[:],
                                func=mybir.ActivationFunctionType.Copy,
                                scale=dv_sb[:, b0 + bl:b0 + bl + 1],
                            )
                        else:
                            for n in range(2):
                                ps = fcps.tile([P, NCLS // 2], dt.float32,
                                               space="PSUM", tag="fc")
                                for k in range(2):
                                    nc.tensor.matmul(
                                        out=ps[:],
                                        lhsT=ht[:, k * P:(k + 1) * P],
                                        rhs=wfc_sb[:, k * NCLS + n * (NCLS // 2):
                                                   k * NCLS + (n + 1) * (NCLS // 2)],
                                        start=(k == 0),
                                        stop=(k == 1),
                                    )
                                if bfc_sb is not None:
                                    fo32 = hfinp.tile([P, NCLS // 2], dt.float32,
                                                      tag="fo32")
                                    nc.vector.tensor_tensor(
                                        out=fo32[:], in0=ps[:],
                                        in1=bfc_sb[:, n * (NCLS // 2):(n + 1) * (NCLS // 2)],
                                        op=mybir.AluOpType.add,
                                    )
                                    nc.vector.tensor_copy(
                                        out=ho[:, bl * NCLS + n * (NCLS // 2):
                                               bl * NCLS + (n + 1) * (NCLS // 2)],
                                        in_=fo32[:],
                                    )
                                else:
                                    nc.vector.tensor_copy(
                                        out=ho[:, bl * NCLS + n * (NCLS // 2):
                                               bl * NCLS + (n + 1) * (NCLS // 2)],
                                        in_=ps[:],
                                    )
                    if not last:
                        nc.sync.dma_start(
                            out=hloc2[b0 * P:(b0 + g) * P, :].rearrange(
                                "(g p) f -> p g f", p=P),
                            in_=ho[:].rearrange("p (g f) -> p g f", g=g),
                        )
                    else:
                        nc.sync.dma_start(
                            out=out[b0 * P:(b0 + g) * P, :].rearrange(
                                "(g p) f -> p g f", p=P),
                            in_=ho[:].rearrange("p (g f) -> p g f", g=g),
                        )

            # ---- layer 1
            layer1_matmul()
            if stop_idx >= 1:
                all_gather(hloc1, hfull1)
            if stop_idx >= 2:
                gather_layer(hfull1, b1_sb, last=False)
            # ---- layer 2
            if stop_idx >= 3:
                all_gather(hloc2, hfull2)
            if stop_idx >= 4:
                gather_layer(hfull2, b2_sb, last=True)
            if debug_dumps:
                hl1d = nc.dram_tensor("hl1d", [SLOTS, HID], dt.bfloat16,
                                      kind="ExternalOutput")
                hl2d = nc.dram_tensor("hl2d", [SLOTS, HID], dt.bfloat16,
                                      kind="ExternalOutput")
                hf1d = nc.dram_tensor("hf1d", [N_CORES * SLOTS, HID], dt.bfloat16,
                                      kind="ExternalOutput")
                nc.sync.dma_start(out=hl1d[:], in_=hloc1[:])
                nc.sync.dma_start(out=hl2d[:], in_=hloc2[:])
                nc.sync.dma_start(out=hf1d[:], in_=hfull1[:])

    nc.compile()
    return nc


def kernel(x, edge_index, W1, b1, W2, b2, Wfc, bfc):
    x = np.asarray(x, dtype=np.float32)
    edge_index = np.asarray(edge_index)
    W1 = np.asarray(W1, dtype=np.float32)
    b1 = np.asarray(b1, dtype=np.float32)
    W2 = np.asarray(W2, dtype=np.float32)
    b2 = np.asarray(b2, dtype=np.float32)
    Wfc = np.asarray(Wfc, dtype=np.float32)
    bfc = np.asarray(bfc, dtype=np.float32)
    plan, in_maps, ids_order = _preprocess(x, edge_index, W1, b1, W2, b2, Wfc, bfc)
    nc = _build_program(plan)
    res = run_bass_kernel_spmd(nc, in_maps, core_ids=list(range(N_CORES)))
    full = np.empty((N_NODES, NCLS), np.float32)
    for c in range(N_CORES):
        full[ids_order[c]] = res.results[c]["out"][: len(ids_order[c])].astype(np.float32)
    return full
